# revision 1
# baseline (speedup 1.0000x reference)
"""OHNM (online hard negative mining) MSE loss on 8 Trainium2 NeuronCores.

Reference computation (per map, maps = character & affinity):
    all_loss = (pred - target)^2            # N = 64*512*512 pixels
    pos_sum  = sum of all_loss * weight     # over pixels with target != 0
    num_pos  = count(target != 0)
    topk     = top-1000 of all_loss over pixels with target == 0
    k        = min(1000, 4*num_pos, num_neg)
    loss     = (pos_sum + sum(topk[:k])) / (num_pos + k)
Result = loss_character + loss_affinity  (f32 scalar).

Sharding: data-parallel over batch, 8 batches per core, processed as 4 merged
[128 x 4096] tiles per map. Per tile:
  ACT   : n = Relu(1 - 1.2*t)  (exact 0/1 negative mask; targets are 0 or >0.9)
          with accum_out = per-partition negative count
  GpSimd: d = pred - target
  ACT   : l = d^2 (in place)
  DVE   : negv = l*n ; lp = l - negv (in place) ; wlp = lp*w (in place)
  ACT   : Identity(wlp) accum -> per-partition positive weighted loss
  DVE   : max8(negv) -> top-8 negative losses per (partition, tile) chunk
Host gathers the 8 cores' partials and does the exact final top-k reduce over
the candidate set. Candidate coverage is exact unless some 4096-element chunk
holds >8 of the global top-1000 (verified on host; falls back to exact numpy
in that astronomically unlikely case).
"""

import sys

sys.path.insert(0, "/opt/trn_rl_repo")

import numpy as np

import concourse.bacc as bacc
import concourse.tile as tile
from concourse import mybir
from concourse.bass_utils import run_bass_kernel_spmd

B, C, H, W = 64, 2, 512, 512
N_CORES = 8
BPC = B // N_CORES  # batches per core
P = 128
F = (H * W) // P  # 2048 elements per partition per batch-map
NTM = BPC  # tiles per map per core (1 batch each)
F2 = F  # free size of a tile
K_MAX = 1000
N_PIX = B * H * W
N_MAP = N_PIX  # pixels per map

_CACHE = {}


def _build_nc():
    f32 = mybir.dt.float32
    bf16 = mybir.dt.bfloat16
    nc = bacc.Bacc()
    pred = nc.declare_dram_parameter("pred", [BPC, C, P, F], f32, isOutput=False)
    cmap = nc.declare_dram_parameter("cmap", [BPC, P, F], f32, isOutput=False)
    amap = nc.declare_dram_parameter("amap", [BPC, P, F], f32, isOutput=False)
    cw = nc.declare_dram_parameter("cw", [BPC, P, F], f32, isOutput=False)
    aw = nc.declare_dram_parameter("aw", [BPC, P, F], f32, isOutput=False)
    cand_o = nc.declare_dram_parameter("cand", [P, 2 * NTM * 8], f32, isOutput=True)
    psum_o = nc.declare_dram_parameter("psums", [P, 2 * NTM], f32, isOutput=True)
    cnt_o = nc.declare_dram_parameter("cnts", [P, 2 * NTM], f32, isOutput=True)

    with tile.TileContext(nc) as tc:
        with (
            tc.tile_pool(name="io", bufs=4) as io,
            tc.tile_pool(name="work", bufs=4) as work,
            tc.tile_pool(name="short", bufs=2) as short,
            tc.tile_pool(name="scr", bufs=1) as scr,
            tc.tile_pool(name="singles", bufs=1) as singles,
        ):
            candt = singles.tile([P, 2 * NTM * 8], f32)
            post = singles.tile([P, 2 * NTM], f32)
            cntt = singles.tile([P, 2 * NTM], f32)

            for m, (tmap, wmap, ch) in enumerate(((cmap, cw, 0), (amap, aw, 1))):
                for bi in range(NTM):
                    j = m * NTM + bi
                    p_t = io.tile([P, F2], f32, tag="p")
                    t_t = io.tile([P, F2], f32, tag="t")
                    w_t = io.tile([P, F2], f32, tag="w")
                    # w first for lead time (it is consumed last but must not
                    # stall the tail of the DVE chain); t rides SWDGE (gpsimd)
                    # to spread queue pressure
                    nc.sync.dma_start(out=w_t, in_=wmap[bi])
                    nc.sync.dma_start(out=p_t, in_=pred[bi, ch])
                    nc.gpsimd.dma_start(out=t_t, in_=tmap[bi])

                    # n = Relu(1 - 1.2*t): exactly 1 at negatives (t == 0),
                    # exactly 0 at positives (t > 0.9); accum = negative count
                    n_t = short.tile([P, F2], bf16, tag="n")
                    nc.scalar.activation(
                        out=n_t,
                        in_=t_t,
                        func=mybir.ActivationFunctionType.Relu,
                        bias=1.0,
                        scale=-1.2,
                        accum_out=cntt[:, j : j + 1],
                    )

                    # w in bf16 so the wlp multiply hits the DVE 2x mode
                    w_b = work.tile([P, F2], bf16, tag="wb")
                    nc.scalar.copy(w_b, w_t)

                    # d = pred - target (f32, short-lived), l = d^2 in bf16
                    # so every following DVE op is pure bf16 (2x-mode eligible)
                    d = short.tile([P, F2], f32, tag="d")
                    nc.gpsimd.tensor_sub(d, p_t, t_t)
                    l_b = work.tile([P, F2], bf16, tag="lb")
                    nc.scalar.square(l_b, d)

                    # negv = l * n (negative-only losses), bf16: exact 0 at
                    # positives; ~0.4% rounding on negatives is harmless (it
                    # only feeds the top-k path and a tiny residual in pos_sum)
                    negv = work.tile([P, F2], bf16, tag="negv")
                    nc.vector.tensor_mul(negv, l_b, n_t)

                    # top-8 negative losses of this chunk (issued early: it
                    # only depends on negv)
                    nc.vector.max(out=candt[:, j * 8 : (j + 1) * 8], in_=negv)

                    # lp = l - negv (exact 0 at negatives: negv == l_b there)
                    lp_b = work.tile([P, F2], bf16, tag="lpb")
                    nc.vector.tensor_sub(lp_b, l_b, negv)
                    wlp_b = short.tile([P, F2], bf16, tag="wlpb")
                    nc.vector.tensor_mul(wlp_b, lp_b, w_b)

                    # per-partition positive weighted sum via ACT accumulator
                    junk = scr.tile([P, F2], bf16, tag="junk")
                    nc.scalar.activation(
                        out=junk,
                        in_=wlp_b,
                        func=mybir.ActivationFunctionType.Identity,
                        accum_out=post[:, j : j + 1],
                    )

            nc.sync.dma_start(out=cand_o[:], in_=candt)
            nc.sync.dma_start(out=psum_o[:], in_=post)
            nc.sync.dma_start(out=cnt_o[:], in_=cntt)
    nc.compile()
    return nc


def _get_nc():
    if "nc" not in _CACHE:
        _CACHE["nc"] = _build_nc()
    return _CACHE["nc"]


def _ohnm_np(pred, target, weight):
    """Exact numpy fallback, mirrors the reference."""
    all_loss = (pred - target) ** 2
    pos_mask = target != 0
    num_pos = int(pos_mask.sum())
    num_neg = pred.size - num_pos
    pos_sum = float((all_loss * weight)[pos_mask].astype(np.float64).sum())
    neg_loss = np.where(pos_mask, -np.inf, all_loss)
    k = min(K_MAX, 4 * num_pos, num_neg)
    topk = np.sort(neg_loss.ravel())[-K_MAX:][::-1]
    neg_sum = float(topk[:k].astype(np.float64).sum())
    return np.float32((pos_sum + neg_sum) / np.float64(num_pos + k))


def _combine_map(results, m):
    """Host-side final reduce for one map from the 8 cores' partials."""
    pos_sum = 0.0
    num_neg = 0.0
    cands = []
    for r in results:
        pos_sum += float(r["psums"][:, m * NTM : (m + 1) * NTM].astype(np.float64).sum())
        num_neg += float(r["cnts"][:, m * NTM : (m + 1) * NTM].astype(np.float64).sum())
        cands.append(r["cand"][:, m * NTM * 8 : (m + 1) * NTM * 8].reshape(P, NTM, 8))
    cand = np.stack(cands)  # [cores, P, NTM, 8] descending within each chunk
    num_neg = int(round(num_neg))
    num_pos = N_MAP - num_neg
    k = min(K_MAX, 4 * num_pos, num_neg)
    flat = np.sort(cand.ravel())[::-1]
    neg_sum = float(flat[:k].astype(np.float64).sum()) if k > 0 else 0.0
    ok = True
    if k > 0:
        tau = flat[k - 1]
        # A chunk can only hide a missed top-k element if its own 8th-largest
        # (the smallest we kept) is strictly above the k-th candidate.
        chunk_min = cand[..., 7]
        ok = not bool((chunk_min > tau).any())
    loss = np.float32((pos_sum + neg_sum) / np.float64(num_pos + k))
    return loss, ok


def kernel(output, character_map, affinity_map, character_weight, affinity_weight):
    output = np.asarray(output, dtype=np.float32)
    character_map = np.asarray(character_map, dtype=np.float32)
    affinity_map = np.asarray(affinity_map, dtype=np.float32)
    character_weight = np.asarray(character_weight, dtype=np.float32)
    affinity_weight = np.asarray(affinity_weight, dtype=np.float32)

    nc = _get_nc()
    in_maps = []
    for i in range(N_CORES):
        sl = slice(i * BPC, (i + 1) * BPC)
        in_maps.append(
            {
                "pred": np.ascontiguousarray(output[sl]).reshape(BPC, C, P, F),
                "cmap": np.ascontiguousarray(character_map[sl]).reshape(BPC, P, F),
                "amap": np.ascontiguousarray(affinity_map[sl]).reshape(BPC, P, F),
                "cw": np.ascontiguousarray(character_weight[sl]).reshape(BPC, P, F),
                "aw": np.ascontiguousarray(affinity_weight[sl]).reshape(BPC, P, F),
            }
        )
    results = run_bass_kernel_spmd(nc, in_maps, list(range(N_CORES))).results

    loss_c, ok_c = _combine_map(results, 0)
    loss_a, ok_a = _combine_map(results, 1)
    if not ok_c:
        flat = output.transpose(0, 2, 3, 1).reshape(-1, C)
        loss_c = _ohnm_np(
            flat[:, 0], character_map.reshape(-1), character_weight.reshape(-1)
        )
    if not ok_a:
        flat = output.transpose(0, 2, 3, 1).reshape(-1, C)
        loss_a = _ohnm_np(
            flat[:, 1], affinity_map.reshape(-1), affinity_weight.reshape(-1)
        )
    return np.array(np.float32(loss_c) + np.float32(loss_a), dtype=np.float32)



# revision 2
# speedup vs baseline: 1.0602x; 1.0602x over previous
"""OHNM (online hard negative mining) MSE loss on 8 Trainium2 NeuronCores.

Reference computation (per map, maps = character & affinity):
    all_loss = (pred - target)^2            # N = 64*512*512 pixels
    pos_sum  = sum of all_loss * weight     # over pixels with target != 0
    num_pos  = count(target != 0)
    topk     = top-1000 of all_loss over pixels with target == 0
    k        = min(1000, 4*num_pos, num_neg)
    loss     = (pos_sum + sum(topk[:k])) / (num_pos + k)
Result = loss_character + loss_affinity  (f32 scalar).

Sharding: data-parallel over batch, 8 batches per core. Inputs are fed to the
device in bf16 (host-side cast; tolerance is 2e-2 and every sum averages the
rounding noise away), which halves HBM traffic -- the kernel is memory-bound.

Per core each map is a [128, 16384] stream processed as 4 tiles of [128, 4096]:
  ACT : n = Relu(1 - 1.2*t)   exact 0/1 negative mask (targets are 0 or >0.9),
        accum_out = per-partition negative count
  DVE : d = p - t                        (scalar_tensor_tensor, 4x mode)
  ACT : l = d^2
  DVE : negv = l*n                       (4x; exact: n is exactly 0 or 1)
  DVE : junkA = w*l,    accum -> sum(w*l)     per partition   (4x)
  DVE : junkB = w*negv, accum -> sum(w*negv)  per partition   (4x)
        pos_sum = sum(w*l) - sum(w*negv): negative terms are bitwise-identical
        products, so they cancel exactly and only positive terms remain.
  DVE : fold = max(negv[:, :2048], negv[:, 2048:])  (4x)
  DVE : top8 = max8(fold) -> 8 candidates per (partition, tile)
Host gathers the 8 cores' partials and does the final top-k reduce over the
candidate set, with an exact-numpy fallback if the candidate set provably
might miss a top-k element.
"""

import sys

sys.path.insert(0, "/opt/trn_rl_repo")

import ml_dtypes
import numpy as np

import concourse.bacc as bacc
import concourse.tile as tile
from concourse import mybir
from concourse.bass_utils import run_bass_kernel_spmd

B, C, H, W = 64, 2, 512, 512
N_CORES = 8
BPC = B // N_CORES  # batches per core
P = 128
FB = (H * W) // P  # 2048 elements per partition per batch-map
FT = 4096  # tile free size (2 batches worth per partition line)
NT = (BPC * FB) // FT  # tiles per map per core = 4
NIT = 2 * NT  # tile iterations per core (both maps) = 8
FTOT = BPC * FB  # 16384 free elements per map per core
K_MAX = 1000
N_MAP = B * H * W  # pixels per map

_CACHE = {}

BF16 = ml_dtypes.bfloat16


def _build_nc():
    f32 = mybir.dt.float32
    bf16 = mybir.dt.bfloat16
    AF = mybir.ActivationFunctionType
    OP = mybir.AluOpType
    nc = bacc.Bacc()
    pred = nc.declare_dram_parameter("pred", [C, P, FTOT], bf16, isOutput=False)
    cmap = nc.declare_dram_parameter("cmap", [P, FTOT], bf16, isOutput=False)
    amap = nc.declare_dram_parameter("amap", [P, FTOT], bf16, isOutput=False)
    cw = nc.declare_dram_parameter("cw", [P, FTOT], bf16, isOutput=False)
    aw = nc.declare_dram_parameter("aw", [P, FTOT], bf16, isOutput=False)
    cand_o = nc.declare_dram_parameter("cand", [P, NIT * 8], f32, isOutput=True)
    suma_o = nc.declare_dram_parameter("suma", [P, NIT], f32, isOutput=True)
    sumb_o = nc.declare_dram_parameter("sumb", [P, NIT], f32, isOutput=True)
    cnt_o = nc.declare_dram_parameter("cnts", [P, NIT], f32, isOutput=True)

    with tile.TileContext(nc) as tc:
        with (
            tc.tile_pool(name="io", bufs=3) as io,
            tc.tile_pool(name="work", bufs=2) as work,
            tc.tile_pool(name="scr", bufs=1) as scr,
            tc.tile_pool(name="singles", bufs=1) as singles,
        ):
            candt = singles.tile([P, NIT * 8], f32)
            sumat = singles.tile([P, NIT], f32)
            sumbt = singles.tile([P, NIT], f32)
            cntt = singles.tile([P, NIT], f32)

            for m, (tmap, wmap) in enumerate(((cmap, cw), (amap, aw))):
                for ti in range(NT):
                    j = m * NT + ti
                    sl = slice(ti * FT, (ti + 1) * FT)
                    p_t = io.tile([P, FT], bf16, tag="p")
                    t_t = io.tile([P, FT], bf16, tag="t")
                    w_t = io.tile([P, FT], bf16, tag="w")
                    # t rides the gpsimd queue so the mask ACT can start
                    # early while p/w stream on the sync queue
                    nc.gpsimd.dma_start(out=t_t, in_=tmap[:, sl])
                    nc.sync.dma_start(out=p_t, in_=pred[m][:, sl])
                    nc.sync.dma_start(out=w_t, in_=wmap[:, sl])

                    # n = Relu(1 - 1.2*t): exactly 1 at negatives (t == 0),
                    # exactly 0 at positives (t > 0.89 even after bf16
                    # rounding); accum = negative count
                    n_t = work.tile([P, FT], bf16, tag="n")
                    nc.scalar.activation(
                        out=n_t,
                        in_=t_t,
                        func=AF.Relu,
                        bias=1.0,
                        scale=-1.2,
                        accum_out=cntt[:, j : j + 1],
                    )

                    # d = p - t (4x-mode STT: (p*1) - t)
                    d_t = work.tile([P, FT], bf16, tag="d")
                    nc.vector.scalar_tensor_tensor(
                        out=d_t, in0=p_t, scalar=1.0, in1=t_t,
                        op0=OP.mult, op1=OP.subtract,
                    )

                    # l = d^2 on ACT (keeps DVE free for the 4x STT chain)
                    l_t = work.tile([P, FT], bf16, tag="l")
                    nc.scalar.square(l_t, d_t)

                    # negv = l*n: exact 0 at positives, exact copy of l at
                    # negatives (n is exactly 1.0 there)
                    negv = work.tile([P, FT], bf16, tag="negv")
                    nc.vector.scalar_tensor_tensor(
                        out=negv, in0=l_t, scalar=1.0, in1=n_t,
                        op0=OP.mult, op1=OP.mult,
                    )

                    # folded negative stream + top-8 per (partition, tile)
                    fold = work.tile([P, FT // 2], bf16, tag="fold")
                    nc.vector.scalar_tensor_tensor(
                        out=fold, in0=negv[:, : FT // 2], scalar=1.0,
                        in1=negv[:, FT // 2 :], op0=OP.mult, op1=OP.max,
                    )
                    nc.vector.max(out=candt[:, j * 8 : (j + 1) * 8], in_=fold)

                    # sum(w*l) and sum(w*negv) per partition; difference is
                    # the positive weighted loss (exact term cancellation)
                    junk = scr.tile([P, FT], bf16, tag="junk")
                    nc.vector.scalar_tensor_tensor(
                        out=junk, in0=w_t, scalar=1.0, in1=l_t,
                        op0=OP.mult, op1=OP.mult,
                        accum_out=sumat[:, j : j + 1],
                    )
                    junk2 = scr.tile([P, FT], bf16, tag="junk2")
                    nc.vector.scalar_tensor_tensor(
                        out=junk2, in0=w_t, scalar=1.0, in1=negv,
                        op0=OP.mult, op1=OP.mult,
                        accum_out=sumbt[:, j : j + 1],
                    )

            nc.sync.dma_start(out=cand_o[:], in_=candt)
            nc.sync.dma_start(out=suma_o[:], in_=sumat)
            nc.sync.dma_start(out=sumb_o[:], in_=sumbt)
            nc.sync.dma_start(out=cnt_o[:], in_=cntt)
    nc.compile()
    return nc


def _get_nc():
    if "nc" not in _CACHE:
        _CACHE["nc"] = _build_nc()
    return _CACHE["nc"]


def _ohnm_np(pred, target, weight):
    """Exact numpy fallback, mirrors the reference."""
    all_loss = (pred - target) ** 2
    pos_mask = target != 0
    num_pos = int(pos_mask.sum())
    num_neg = pred.size - num_pos
    pos_sum = float((all_loss * weight)[pos_mask].astype(np.float64).sum())
    neg_loss = np.where(pos_mask, -np.inf, all_loss)
    k = min(K_MAX, 4 * num_pos, num_neg)
    topk = np.sort(neg_loss.ravel())[-K_MAX:][::-1]
    neg_sum = float(topk[:k].astype(np.float64).sum())
    return np.float32((pos_sum + neg_sum) / np.float64(num_pos + k))


def _to_core_layout(arr_core):
    """[BPC, H, W] (or already [BPC, P*FB]) f32 -> [P, FTOT] bf16 with each
    partition holding BPC contiguous per-batch segments."""
    a = arr_core.reshape(BPC, P, FB).transpose(1, 0, 2).reshape(P, FTOT)
    return np.ascontiguousarray(a.astype(BF16))


def _combine_map(results, m):
    """Host-side final reduce for one map from the 8 cores' partials."""
    pos_sum = 0.0
    num_neg = 0.0
    cands = []
    for r in results:
        js = slice(m * NT, (m + 1) * NT)
        suma = np.asarray(r["suma"])[:, js].astype(np.float64)
        sumb = np.asarray(r["sumb"])[:, js].astype(np.float64)
        pos_sum += float(suma.sum() - sumb.sum())
        num_neg += float(np.asarray(r["cnts"])[:, js].astype(np.float64).sum())
        cands.append(
            np.asarray(r["cand"])[:, m * NT * 8 : (m + 1) * NT * 8]
            .astype(np.float32)
            .reshape(P, NT, 8)
        )
    cand = np.stack(cands)  # [cores, P, NT, 8] descending within each chunk
    num_neg = int(round(num_neg))
    num_pos = N_MAP - num_neg
    k = min(K_MAX, 4 * num_pos, num_neg)
    flat = np.sort(cand.ravel())[::-1]
    neg_sum = float(flat[:k].astype(np.float64).sum()) if k > 0 else 0.0
    ok = True
    if k > 0:
        tau = flat[k - 1]
        # A chunk can only hide a missed top-k element if its own 8th-largest
        # (the smallest we kept) is strictly above the k-th candidate.
        chunk_min = cand[..., 7]
        ok = not bool((chunk_min > tau).any())
    loss = np.float32((pos_sum + neg_sum) / np.float64(num_pos + k))
    return loss, ok


def kernel(output, character_map, affinity_map, character_weight, affinity_weight):
    output = np.asarray(output, dtype=np.float32)
    character_map = np.asarray(character_map, dtype=np.float32)
    affinity_map = np.asarray(affinity_map, dtype=np.float32)
    character_weight = np.asarray(character_weight, dtype=np.float32)
    affinity_weight = np.asarray(affinity_weight, dtype=np.float32)

    nc = _get_nc()
    in_maps = []
    for i in range(N_CORES):
        sl = slice(i * BPC, (i + 1) * BPC)
        pred_core = np.stack(
            [
                _to_core_layout(output[sl, 0]),
                _to_core_layout(output[sl, 1]),
            ]
        )
        in_maps.append(
            {
                "pred": pred_core,
                "cmap": _to_core_layout(character_map[sl]),
                "amap": _to_core_layout(affinity_map[sl]),
                "cw": _to_core_layout(character_weight[sl]),
                "aw": _to_core_layout(affinity_weight[sl]),
            }
        )
    results = run_bass_kernel_spmd(nc, in_maps, list(range(N_CORES))).results

    loss_c, ok_c = _combine_map(results, 0)
    loss_a, ok_a = _combine_map(results, 1)
    if not ok_c:
        flat = output.transpose(0, 2, 3, 1).reshape(-1, C)
        loss_c = _ohnm_np(
            flat[:, 0], character_map.reshape(-1), character_weight.reshape(-1)
        )
    if not ok_a:
        flat = output.transpose(0, 2, 3, 1).reshape(-1, C)
        loss_a = _ohnm_np(
            flat[:, 1], affinity_map.reshape(-1), affinity_weight.reshape(-1)
        )
    return np.array(np.float32(loss_c) + np.float32(loss_a), dtype=np.float32)


# revision 6
# speedup vs baseline: 1.9354x; 1.8255x over previous
"""OHNM (online hard negative mining) MSE loss on 8 Trainium2 NeuronCores.

Reference computation (per map, maps = character & affinity):
    all_loss = (pred - target)^2            # N = 64*512*512 pixels
    pos_sum  = sum of all_loss * weight     # over pixels with target != 0
    num_pos  = count(target != 0)
    topk     = top-1000 of all_loss over pixels with target == 0
    k        = min(1000, 4*num_pos, num_neg)
    loss     = (pos_sum + sum(topk[:k])) / (num_pos + k)
Result = loss_character + loss_affinity  (f32 scalar).

Sharding: data-parallel over batch, 8 batches per core. Inputs are fed to the
device in bf16 (host-side cast; tolerance is 2e-2 and every sum averages the
rounding noise away), which halves HBM traffic -- the kernel is memory-bound.

Per core each map is a [128, 16384] stream processed as 4 tiles of [128, 4096]:
  ACT : n = Relu(1 - 1.2*t)   exact 0/1 negative mask (targets are 0 or >0.9),
        accum_out = per-partition negative count
  DVE : d = p - t             (tensor_tensor, bf16 2x mode)
  ACT : l = d^2
  DVE : negv = l*n            (2x; exact: n is exactly 0 or 1)
  DVE : top8 = max8(negv) -> 8 candidates per (partition, tile)
  PE  : psumA += w_blk^T @ l_blk,  psumB += w_blk^T @ negv_blk
        (32 128x128 blocks per tile, accumulated across the map's 4 tiles;
        diag(psumA) - diag(psumB) = per-column sum of w*l over positives:
        negative-pixel products are bitwise identical and cancel exactly)
Host gathers the 8 cores' partials (trace of psumA/psumB, counts, candidates)
and does the final top-k reduce over the candidate set, with an exact-numpy
fallback if the candidate set provably might miss a top-k element.
"""

import sys

sys.path.insert(0, "/opt/trn_rl_repo")

import ml_dtypes
import numpy as np

import concourse.bacc as bacc
import concourse.tile as tile
from concourse import mybir
from concourse.bass_utils import run_bass_kernel_spmd

B, C, H, W = 64, 2, 512, 512
N_CORES = 8
BPC = B // N_CORES  # batches per core
P = 128
FB = (H * W) // P  # 2048 elements per partition per batch-map
FT = 4096  # tile free size (2 batches worth per partition line)
NT = (BPC * FB) // FT  # tiles per map per core = 4
NIT = 2 * NT  # tile iterations per core (both maps) = 8
NBLK = FT // P  # 128-col blocks per tile = 32
FTOT = BPC * FB  # 16384 free elements per map per core
K_MAX = 1000
N_MAP = B * H * W  # pixels per map

_CACHE = {}

BF16 = ml_dtypes.bfloat16


def _build_nc():
    f32 = mybir.dt.float32
    bf16 = mybir.dt.bfloat16
    AF = mybir.ActivationFunctionType
    nc = bacc.Bacc()
    pred = nc.declare_dram_parameter("pred", [C, P, FTOT], bf16, isOutput=False)
    cmap = nc.declare_dram_parameter("cmap", [P, FTOT], bf16, isOutput=False)
    amap = nc.declare_dram_parameter("amap", [P, FTOT], bf16, isOutput=False)
    cw = nc.declare_dram_parameter("cw", [P, FTOT], bf16, isOutput=False)
    aw = nc.declare_dram_parameter("aw", [P, FTOT], bf16, isOutput=False)
    cand_o = nc.declare_dram_parameter("cand", [P, NIT * 8], f32, isOutput=True)
    suma_o = nc.declare_dram_parameter("suma", [P, C, P], f32, isOutput=True)
    sumb_o = nc.declare_dram_parameter("sumb", [P, C, P], f32, isOutput=True)
    cnt_o = nc.declare_dram_parameter("cnts", [P, NIT], f32, isOutput=True)

    with tile.TileContext(nc) as tc:
        with (
            tc.tile_pool(name="io", bufs=3) as io,
            tc.tile_pool(name="work", bufs=2) as work,
            tc.tile_pool(name="psum", bufs=1, space="PSUM") as psum,
            tc.tile_pool(name="singles", bufs=1) as singles,
        ):
            candt = singles.tile([P, NIT * 8], f32)
            cntt = singles.tile([P, NIT], f32)
            suma_s = singles.tile([P, C, P], f32)
            sumb_s = singles.tile([P, C, P], f32)
            psA = [
                psum.tile([P, P], f32, tag=f"psA{m}", name=f"psA{m}")
                for m in range(2)
            ]
            psB = [
                psum.tile([P, P], f32, tag=f"psB{m}", name=f"psB{m}")
                for m in range(2)
            ]

            for m, (tmap, wmap) in enumerate(((cmap, cw), (amap, aw))):
                for ti in range(NT):
                    j = m * NT + ti
                    sl = slice(ti * FT, (ti + 1) * FT)
                    p_t = io.tile([P, FT], bf16, tag="p")
                    t_t = io.tile([P, FT], bf16, tag="t")
                    w_t = io.tile([P, FT], bf16, tag="w")
                    # t rides the gpsimd queue so the mask ACT can start
                    # early while p/w stream on the sync queue
                    nc.gpsimd.dma_start(out=t_t, in_=tmap[:, sl])
                    nc.sync.dma_start(out=p_t, in_=pred[m][:, sl])
                    nc.sync.dma_start(out=w_t, in_=wmap[:, sl])

                    # n = Relu(1 - 1.2*t): exactly 1 at negatives (t == 0),
                    # exactly 0 at positives (t > 0.89 even after bf16
                    # rounding); accum = negative count
                    n_t = work.tile([P, FT], bf16, tag="n")
                    nc.scalar.activation(
                        out=n_t,
                        in_=t_t,
                        func=AF.Relu,
                        bias=1.0,
                        scale=-1.2,
                        accum_out=cntt[:, j : j + 1],
                    )

                    # d = p - t  (bf16 tensor_tensor, 2x mode)
                    d_t = work.tile([P, FT], bf16, tag="d")
                    nc.vector.tensor_sub(d_t, p_t, t_t)

                    # l = d^2 on ACT
                    l_t = work.tile([P, FT], bf16, tag="l")
                    nc.scalar.square(l_t, d_t)

                    # negv = l*n: exact 0 at positives, exact copy of l at
                    # negatives (n is exactly 1.0 there)
                    negv = work.tile([P, FT], bf16, tag="negv")
                    nc.vector.tensor_mul(negv, l_t, n_t)

                    # top-8 negative losses per (partition, tile)
                    nc.vector.max(out=candt[:, j * 8 : (j + 1) * 8], in_=negv)

                    # PE: accumulate w^T @ l and w^T @ negv in 128x128 blocks;
                    # only the diagonals are used (per-column dot products)
                    for bk in range(NBLK):
                        bsl = slice(bk * P, (bk + 1) * P)
                        nc.tensor.matmul(
                            psA[m],
                            w_t[:, bsl],
                            l_t[:, bsl],
                            start=ti == 0 and bk == 0,
                            stop=ti == NT - 1 and bk == NBLK - 1,
                        )
                    for bk in range(NBLK):
                        bsl = slice(bk * P, (bk + 1) * P)
                        nc.tensor.matmul(
                            psB[m],
                            w_t[:, bsl],
                            negv[:, bsl],
                            start=ti == 0 and bk == 0,
                            stop=ti == NT - 1 and bk == NBLK - 1,
                        )

            for m in range(2):
                nc.vector.tensor_copy(suma_s[:, m], psA[m])
                nc.vector.tensor_copy(sumb_s[:, m], psB[m])

            nc.sync.dma_start(out=cand_o[:], in_=candt)
            nc.sync.dma_start(out=suma_o[:], in_=suma_s)
            nc.sync.dma_start(out=sumb_o[:], in_=sumb_s)
            nc.sync.dma_start(out=cnt_o[:], in_=cntt)
    nc.compile()
    return nc


def _get_nc():
    if "nc" not in _CACHE:
        _CACHE["nc"] = _build_nc()
    return _CACHE["nc"]


def _ohnm_np(pred, target, weight):
    """Exact numpy fallback, mirrors the reference."""
    all_loss = (pred - target) ** 2
    pos_mask = target != 0
    num_pos = int(pos_mask.sum())
    num_neg = pred.size - num_pos
    pos_sum = float((all_loss * weight)[pos_mask].astype(np.float64).sum())
    neg_loss = np.where(pos_mask, -np.inf, all_loss)
    k = min(K_MAX, 4 * num_pos, num_neg)
    topk = np.sort(neg_loss.ravel())[-K_MAX:][::-1]
    neg_sum = float(topk[:k].astype(np.float64).sum())
    return np.float32((pos_sum + neg_sum) / np.float64(num_pos + k))


def _to_core_layout(arr_core):
    """[BPC, H, W] f32 -> [P, FTOT] bf16 with each partition holding BPC
    contiguous per-batch segments."""
    a = arr_core.reshape(BPC, P, FB).transpose(1, 0, 2).reshape(P, FTOT)
    return np.ascontiguousarray(a.astype(BF16))


def _combine_map(results, m):
    """Host-side final reduce for one map from the 8 cores' partials."""
    pos_sum = 0.0
    num_neg = 0.0
    cands = []
    for r in results:
        js = slice(m * NT, (m + 1) * NT)
        da = np.diagonal(np.asarray(r["suma"])[:, m]).astype(np.float64)
        db = np.diagonal(np.asarray(r["sumb"])[:, m]).astype(np.float64)
        pos_sum += float(da.sum() - db.sum())
        num_neg += float(np.asarray(r["cnts"])[:, js].astype(np.float64).sum())
        cands.append(
            np.asarray(r["cand"])[:, m * NT * 8 : (m + 1) * NT * 8]
            .astype(np.float32)
            .reshape(P, NT, 8)
        )
    cand = np.stack(cands)  # [cores, P, NT, 8] descending within each chunk
    num_neg = int(round(num_neg))
    num_pos = N_MAP - num_neg
    k = min(K_MAX, 4 * num_pos, num_neg)
    flat = np.sort(cand.ravel())[::-1]
    neg_sum = float(flat[:k].astype(np.float64).sum()) if k > 0 else 0.0
    ok = True
    if k > 0:
        tau = flat[k - 1]
        # A chunk can only hide a missed top-k element if its own 8th-largest
        # (the smallest we kept) is strictly above the k-th candidate.
        chunk_min = cand[..., 7]
        ok = not bool((chunk_min > tau).any())
    loss = np.float32((pos_sum + neg_sum) / np.float64(num_pos + k))
    return loss, ok


def kernel(output, character_map, affinity_map, character_weight, affinity_weight):
    output = np.asarray(output, dtype=np.float32)
    character_map = np.asarray(character_map, dtype=np.float32)
    affinity_map = np.asarray(affinity_map, dtype=np.float32)
    character_weight = np.asarray(character_weight, dtype=np.float32)
    affinity_weight = np.asarray(affinity_weight, dtype=np.float32)

    nc = _get_nc()
    in_maps = []
    for i in range(N_CORES):
        sl = slice(i * BPC, (i + 1) * BPC)
        pred_core = np.stack(
            [
                _to_core_layout(output[sl, 0]),
                _to_core_layout(output[sl, 1]),
            ]
        )
        in_maps.append(
            {
                "pred": pred_core,
                "cmap": _to_core_layout(character_map[sl]),
                "amap": _to_core_layout(affinity_map[sl]),
                "cw": _to_core_layout(character_weight[sl]),
                "aw": _to_core_layout(affinity_weight[sl]),
            }
        )
    results = run_bass_kernel_spmd(nc, in_maps, list(range(N_CORES))).results

    loss_c, ok_c = _combine_map(results, 0)
    loss_a, ok_a = _combine_map(results, 1)
    if not ok_c:
        flat = output.transpose(0, 2, 3, 1).reshape(-1, C)
        loss_c = _ohnm_np(
            flat[:, 0], character_map.reshape(-1), character_weight.reshape(-1)
        )
    if not ok_a:
        flat = output.transpose(0, 2, 3, 1).reshape(-1, C)
        loss_a = _ohnm_np(
            flat[:, 1], affinity_map.reshape(-1), affinity_weight.reshape(-1)
        )
    return np.array(np.float32(loss_c) + np.float32(loss_a), dtype=np.float32)
